# revision 1
# baseline (speedup 1.0000x reference)
"""Slot-attention (softmax over queries + key renormalization) on 8 TRN2 NeuronCores.

Sharding: data-parallel over batch (b=8 -> one batch element per core, no
collectives). Per-core fused kernel:

  xT/cT/W*T via PE transposes (f32 in), rounded to bf16 on the PSUM->SBUF copy
  v  = cT^T @ WvT                      [m, e] (needed by every head -> first)
  per head pair et2 (projections interleaved with the head loop):
    qT[et2] = WqT^T @ xT, kT[et2] = WkT^T @ cT    [e, n] / [e, m]
    per head h in pair:
      simT[j, i] = khT^T @ qhT                    (PSUM f32, bf16 operands)
      expT[j, i] = exp(scale*simT)                (ACT, fused accum -> sT[j])
      invS = 1/sT; vhs = [v*invS | invS]          (DVE + GPSIMD)
      av[0:65, i] = vhs^T @ expT                  (row 64 = renormalizer r[i])
  per half (heads 0-3 / 4-7):
    invr = 1/r   (DMA-gather partition-striped r_stage, DVE reciprocal)
    outT = outTu * bcast(invr)                    (selector-matmul broadcast)
    y   += outT^T @ WoT (+ bo)                    (SBUF-accumulated halves)

Matmul operands bf16 (1 cyc/row on PE), accumulation f32 in PSUM,
softmax statistics f32.
"""

import dataclasses
import os
import sys

sys.path.insert(0, "/opt/trn_rl_repo")

import numpy as np

import concourse.bass as bass
import concourse.mybir as mybir
import concourse.tile as tile
from concourse import bacc
from concourse.bass_utils import run_bass_kernel_spmd
from concourse.masks import make_identity

B = 8
N = 1024  # queries
M = 1024  # keys
D = 512   # model dim
H = 8
DH = 64
INNER = H * DH
SCALE = DH ** -0.5
P = 128

F32 = mybir.dt.float32
CDT = mybir.dt.bfloat16

B0 = int(os.environ.get("B0", "7"))
_DRAINED = {}
B1 = int(os.environ.get("B1", "2"))
YB = int(os.environ.get("YB", "4"))
IB = int(os.environ.get("IB", "2"))
SB = int(os.environ.get("SB", "2"))
import json as _json
BUMP = {int(k): v for k, v in _json.loads(os.environ.get("BUMP", '{"1": 1}')).items()}

Exp = mybir.ActivationFunctionType.Exp
Mult = mybir.AluOpType.mult
Add = mybir.AluOpType.add


def build(nc: bass.Bass):
    _DRAINED.clear()
    x_d = nc.declare_dram_parameter("x", [N, D], F32, isOutput=False)
    c_d = nc.declare_dram_parameter("context", [M, D], F32, isOutput=False)
    wq_d = nc.declare_dram_parameter("Wq", [INNER, D], F32, isOutput=False)
    wk_d = nc.declare_dram_parameter("Wk", [INNER, D], F32, isOutput=False)
    wv_d = nc.declare_dram_parameter("Wv", [INNER, D], F32, isOutput=False)
    wo_d = nc.declare_dram_parameter("Wo", [D, INNER], F32, isOutput=False)
    bo_d = nc.declare_dram_parameter("bo", [D], F32, isOutput=False)
    out_d = nc.declare_dram_parameter("out", [N, D], F32, isOutput=True)

    with tile.TileContext(nc) as tc:
        with tc.tile_pool(name="const", bufs=1) as const:
            ident = const.tile([P, P], F32, tag="ident")
            make_identity(nc, ident[:, :])
            ones128 = const.tile([1, P], CDT, tag="ones128")
            nc.gpsimd.memset(ones128[:, :], 1.0)
            # One-row masks: sel_lo = [1s x64 | 0s x64], sel_hi = [0|1].
            # Two accumulated K=1 matmuls broadcast each head's inv-renorm row
            # into its 64-partition half of a [128, 512] psum tile.
            sel_lo = const.tile([1, P], CDT, tag="sel_lo")
            nc.gpsimd.memset(sel_lo[:, 0:DH], 1.0)
            nc.gpsimd.memset(sel_lo[:, DH:P], 0.0)
            sel_hi = const.tile([1, P], CDT, tag="sel_hi")
            nc.gpsimd.memset(sel_hi[:, 0:DH], 0.0)
            nc.gpsimd.memset(sel_hi[:, DH:P], 1.0)
            bo_s = const.tile([1, D], F32, tag="bo_s")
            bo_sb = const.tile([1, D], CDT, tag="bo_sb")
            bo_b = const.tile([P, D], F32, tag="bo_b")

            with tc.tile_pool(name="stage", bufs=1) as stage:
                wT = {n: stage.tile([P, 4 * INNER], CDT, tag=f"w{n}T", name=f"w{n}T")
                      for n in ("q", "k", "v")}
                woT = stage.tile([P, 4 * D], CDT, tag="woT")
                xT = stage.tile([P, 4 * N], CDT, tag="xT")
                cT = stage.tile([P, 4 * M], CDT, tag="cT")
                v = [stage.tile([P, INNER], CDT, tag=f"v{t}", name=f"v{t}")
                     for t in range(8)]

                with tc.tile_pool(name="outp", bufs=1) as outp:
                    qT = [outp.tile([P, N], CDT, tag=f"qT{t}", name=f"qT{t}") for t in range(4)]
                    kT = [outp.tile([P, M], CDT, tag=f"kT{t}", name=f"kT{t}") for t in range(4)]
                    outTu = [outp.tile([P, N], F32, tag=f"outTu{t}", name=f"outTu{t}") for t in range(4)]
                    outT = [outp.tile([P, N], CDT, tag=f"outT{t}", name=f"outT{t}") for t in range(4)]
                    y_acc = [outp.tile([P, D], F32, tag=f"y_acc{t}", name=f"y_acc{t}")
                             for t in range(8)]

                    with tc.tile_pool(name="head", bufs=1) as head, \
                         tc.tile_pool(name="norm", bufs=1) as norm, \
                         tc.tile_pool(name="ld", bufs=1) as ld:
                        ps_at = tc.alloc_tile_pool(name="ps_at", bufs=1, space="PSUM")
                        _phase_prep(nc, tc, ld, ps_at, x_d, c_d, wq_d, wk_d,
                                    ident, wT, xT, cT, qT, kT)
                        fillers = [
                            ("vload", _load_w_gen(nc, ld, ps_at, "v", wv_d,
                                                  ident, wT, tag="ps_misc", bufs=2)),
                            ("vproj", _v_proj_gen(nc, ps_at, cT, wT, v)),
                            ("proj1", _project_qk(nc, ps_at, 1, wT, xT, cT, qT, kT)),
                            ("wo", _wo_gen(nc, ld, ps_at, wo_d, bo_d, bo_s,
                                           ident, woT, ones128, bo_sb, bo_b)),
                            ("proj2", _project_qk(nc, ps_at, 2, wT, xT, cT, qT, kT)),
                            ("proj3", _project_qk(nc, ps_at, 3, wT, xT, cT, qT, kT)),
                        ]
                        prev = None
                        invr = {}
                        for h in range(H):
                            prev = _head_step(nc, head, norm, ps_at, h, prev,
                                              qT, kT, v, outTu, invr, fillers)
                            if h >= 2 and (h - 1) % 2 == 1:
                                # woT/bo_b safety: post gens sit behind "wo"
                                # in the FIFO queue, so their instructions are
                                # emitted only after wo's - no force needed
                                fillers.append((f"post{h - 1}", _post_head(
                                    nc, norm, ps_at, h - 1, outTu, outT,
                                    invr, sel_lo, sel_hi, woT, bo_b,
                                    y_acc, out_d)))
                        for pair in fillers:
                            _drain_gen(pair[1])
                        _tail(nc, head, norm, ps_at, prev, v, outTu, outT,
                              invr, sel_lo, sel_hi, woT, y_acc, out_d)
                        ps_at.release()
    return nc


def _phase_prep(nc, tc, ld, ps_at, x_d, c_d, wq_d, wk_d,
                ident, wT, xT, cT, qT, kT):
    """Critical path to the first exp: Wq/Wk, then x (-> full qT[0]), then
    context tiles streamed through transpose + chunked kT[0] projection so
    head 0's sims start as soon as the first context chunks land."""
    for wname, wd in (("q", wq_d), ("k", wk_d)):
        _drain_gen(_load_w_gen(nc, ld, ps_at, wname, wd, ident, wT,
                               tag="ps_misc", bufs=2))
    for nt in range(8):
        sb = ld.tile([P, D], F32, tag="xld", bufs=6, name=f"x{nt}")
        nc.sync.dma_start(sb[:, :], x_d[nt * P:(nt + 1) * P, :])
        pt = ps_at.tile([P, D], F32, tag="ps_sim", bufs=2, name=f"ptxx{nt}")
        for dt_ in range(4):
            nc.tensor.transpose(pt[:, dt_ * P:(dt_ + 1) * P],
                                sb[:, dt_ * P:(dt_ + 1) * P], ident[:, :])
        dst = xT[:, :].rearrange("p (a b) -> p a b", a=4)[:, :, nt * P:(nt + 1) * P]
        srcp = pt[:, :].rearrange("p (a b) -> p a b", a=4)
        if nt % 2 == 0:
            nc.scalar.copy(dst, srcp)
        else:
            nc.vector.tensor_copy(dst, srcp)
    # context: stream 2-tile chunks through transpose + kT[0] chunk
    # projection; the full qT[0] projection is interleaved after chunk 1 so
    # neither blocks the other on the in-order PE stream
    for ct in range(4):
        if ct == 1:
            for ic in range(2):
                pq = ps_at.tile([P, 512], F32, tag="ps_sim", bufs=2, name=f"pq0_{ic}")
                for dt_ in range(4):
                    nc.tensor.matmul(
                        pq[:, :],
                        wT["q"][:, dt_ * INNER: dt_ * INNER + P],
                        xT[:, dt_ * N + ic * 512: dt_ * N + (ic + 1) * 512],
                        start=(dt_ == 0), stop=(dt_ == 3))
                nc.scalar.copy(qT[0][:, ic * 512:(ic + 1) * 512], pq[:, :])
        for t in (2 * ct, 2 * ct + 1):
            sb = ld.tile([P, D], F32, tag="xld", bufs=6, name=f"c{t}")
            nc.sync.dma_start(sb[:, :], c_d[t * P:(t + 1) * P, :])
            pt = ps_at.tile([P, D], F32, tag="ps_misc", bufs=2, name=f"ptxc{t}")
            for dt_ in range(4):
                nc.tensor.transpose(pt[:, dt_ * P:(dt_ + 1) * P],
                                    sb[:, dt_ * P:(dt_ + 1) * P], ident[:, :])
            dst = cT[:, :].rearrange("p (a b) -> p a b", a=4)[:, :, t * P:(t + 1) * P]
            srcp = pt[:, :].rearrange("p (a b) -> p a b", a=4)
            nc.vector.tensor_copy(dst, srcp)
        pk = ps_at.tile([P, 256], F32, tag="ps_av", bufs=1, name=f"pk0_{ct}")
        for dt_ in range(4):
            nc.tensor.matmul(
                pk[:, :],
                wT["k"][:, dt_ * INNER: dt_ * INNER + P],
                cT[:, dt_ * M + ct * 256: dt_ * M + (ct + 1) * 256],
                start=(dt_ == 0), stop=(dt_ == 3))
        nc.vector.tensor_copy(kT[0][:, ct * 256:(ct + 1) * 256], pk[:, :])


def _load_w_gen(nc, ld, ps_psum, wname, wd, ident, wT, tag="ps_w", bufs=None):
    for et in range(4):
        wsb = ld.tile([P, D], F32, tag="wld", bufs=6, name=f"w{wname}{et}")
        nc.sync.dma_start(wsb[:, :], wd[et * P:(et + 1) * P, :])
        kw = {} if bufs is None else {"bufs": bufs}
        pt = ps_psum.tile([P, D], F32, tag=tag, name=f"ptw{wname}{et}", **kw)
        for dt_ in range(4):
            nc.tensor.transpose(pt[:, dt_ * P:(dt_ + 1) * P],
                                wsb[:, dt_ * P:(dt_ + 1) * P], ident[:, :])
            if dt_ == 3:
                dst = wT[wname][:, :].rearrange("p (a b) -> p a b", a=4)[:, :, et * P:(et + 1) * P]
                srcp = pt[:, :].rearrange("p (a b) -> p a b", a=4)
                nc.vector.tensor_copy(dst, srcp)
            yield


def _v_proj_gen(nc, ps_at, cT, wT, v):
    """v[mt] = cT^T @ WvT, one matmul per yield."""
    for mt in range(8):
        pv = ps_at.tile([P, INNER], F32, tag="ps_misc", bufs=2, name=f"pv{mt}")
        for dt_ in range(4):
            nc.tensor.matmul(
                pv[:, :],
                cT[:, dt_ * M + mt * P:dt_ * M + (mt + 1) * P],
                wT["v"][:, dt_ * INNER:(dt_ + 1) * INNER],
                start=(dt_ == 0), stop=(dt_ == 3))
            if dt_ == 3:
                nc.vector.tensor_copy(v[mt][:, :], pv[:, :])
            yield


def _wo_gen(nc, ld, ps_at, wo_d, bo_d, bo_s, ident, woT, ones128, bo_sb, bo_b):
    nc.sync.dma_start(bo_s[:, :], bo_d[None, :])
    nc.vector.tensor_copy(bo_sb[:, :], bo_s[:, :])
    wosb = [ld.tile([P, INNER], F32, tag=f"wo{dt_}", name=f"wo{dt_}") for dt_ in range(4)]
    for dt_ in range(4):
        nc.sync.dma_start(wosb[dt_][:, :], wo_d[dt_ * P:(dt_ + 1) * P, :])
    for et in range(4):
        pt = ps_at.tile([P, D], F32, tag="ps_misc", bufs=2, name=f"ptwo{et}")
        for dt_ in range(4):
            nc.tensor.transpose(pt[:, dt_ * P:(dt_ + 1) * P],
                                wosb[dt_][:, et * P:(et + 1) * P], ident[:, :])
            if dt_ == 3:
                nc.vector.tensor_copy(woT[:, et * D:(et + 1) * D], pt[:, :])
            yield
    pbo = ps_at.tile([P, D], F32, tag="ps_misc", bufs=2, name="pbo")
    nc.tensor.matmul(pbo[:, :], ones128[:, :], bo_sb[:, :], start=True, stop=True)
    nc.scalar.copy(bo_b[:, :], pbo[:, :])
    yield


def _project_qk(nc, ps_pj, et, wT, xT, cT, qT, kT):
    """Generator: yields after each PE matmul so the caller can interleave the
    next pair's q/k projections into the current pair's head loop."""
    for ic in range(2):
        for nm, srcT, dstT, NN in (("q", xT, qT, N), ("k", cT, kT, M)):
            pp = ps_pj.tile([P, 512], F32, tag="ps_misc", bufs=2,
                            name=f"p{nm}{et}_{ic}")
            for dt_ in range(4):
                nc.tensor.matmul(
                    pp[:, :],
                    wT[nm][:, dt_ * INNER + et * P: dt_ * INNER + (et + 1) * P],
                    srcT[:, dt_ * NN + ic * 512: dt_ * NN + (ic + 1) * 512],
                    start=(dt_ == 0), stop=(dt_ == 3))
                if dt_ == 3:
                    nc.vector.tensor_copy(dstT[et][:, ic * 512:(ic + 1) * 512], pp[:, :])
                yield


def _drain_gen(g):
    if g is not None:
        for _ in g:
            pass


def _head_step(nc, head, norm, ps_at, h, prev, qT, kT, v, outTu, invr, fillers):
    """Emit sim+exp for head h, interleaving the previous head's attn@v
    per j-tile. The previous head's vhs tiles are built lazily right before
    their attn@v matmul so the v projection can trail into this head."""
    DEADLINES = {1: ("vload",), 2: ("vproj", "proj1"),
                 4: ("proj2",), 6: ("proj3",)}
    if h is not None:
        for need in DEADLINES.get(h, ()):
            for pair in list(fillers):
                if pair[0] == need:
                    _drain_gen(pair[1])
                    fillers.remove(pair)
    if h is not None:
        et2, ro = h // 2, (h % 2) * 64
        sT = head.tile([P, 8], F32, tag="sT", bufs=SB, name=f"sT{h}")
        invS = head.tile([P, 8], F32, tag="invS", bufs=SB, name=f"invS{h}")
        expT = []
        own_vhs = []
    for jt in range(8):
        if h is not None:
            # head 0: the ps_av bank is idle until pav(0) exists, so odd
            # j-tiles borrow it as a third sim buffer to deepen the
            # sim->exp pipeline during ramp-up
            if h == 0 and jt % 2 == 1:
                psim = ps_at.tile([P, N], F32, tag="ps_av", bufs=1, name=f"psim{h}_{jt}")
            else:
                psim = ps_at.tile([P, N], F32, tag="ps_sim", bufs=2, name=f"psim{h}_{jt}")
            for ic in range(2):
                nc.tensor.matmul(
                    psim[:, ic * 512:(ic + 1) * 512],
                    kT[et2][ro:ro + 64, jt * P:(jt + 1) * P],
                    qT[et2][ro:ro + 64, ic * 512:(ic + 1) * 512],
                    start=True, stop=True)
            eT = head.tile([P, N], CDT, tag=f"expT{jt}", bufs=2, name=f"expT{h}_{jt}")
            nc.scalar.activation(eT[:, :], psim[:, :], Exp, scale=SCALE,
                                 accum_out=sT[:, jt:jt + 1])
            expT.append(eT)
        if prev is not None:
            ph, pav, pexp, pinvS, pvhs = prev
            if ph == 0:
                # vhs(0, jt) reads v[jt]: its projection (4 MMs per m-tile,
                # in order) must be EMITTED first; drain the filler queue
                # until group jt is out (vload is already forced done)
                while fillers and fillers[0][0] == "vproj" and \
                        _DRAINED.get("vproj", 0) < 4 * (jt + 1):
                    try:
                        next(fillers[0][1])
                        _DRAINED["vproj"] = _DRAINED.get("vproj", 0) + 1
                    except StopIteration:
                        fillers.pop(0)
            pvhs.append(_make_vhs(nc, head, ph, jt, v, pinvS))
            for ic in range(2):
                nc.tensor.matmul(pav[:, ic * 512:(ic + 1) * 512],
                                 pvhs[jt][:, :],
                                 pexp[jt][:, ic * 512:(ic + 1) * 512],
                                 start=(jt == 0), stop=(jt == 7))
        budget = B0 if prev is None else (B1 + BUMP.get(h, 0))
        while budget > 0 and fillers:
            try:
                next(fillers[0][1])
                _DRAINED[fillers[0][0]] = _DRAINED.get(fillers[0][0], 0) + 1
                budget -= 1
            except StopIteration:
                fillers.pop(0)
        if h == H - 1:
            # last head: per-column reciprocal + vhs at slot end, so the
            # tail's attn@v is gated per j-tile instead of on the full sT
            nc.vector.reciprocal(invS[:, jt:jt + 1], sT[:, jt:jt + 1])
            own_vhs.append(_make_vhs(nc, head, h, jt, v, invS,
                                     eng="dve" if jt % 2 else "pool"))
    if prev is not None:
        ph, pav_p, _, _, _ = prev
        pet2, pro = ph // 2, (ph % 2) * 64
        nc.vector.tensor_copy(outTu[pet2][pro:pro + 64, :], pav_p[0:DH, :])
        iv = norm.tile([1, N], CDT, tag=f"invr{ph % 2}", bufs=2, name=f"invr{ph}")
        with nc.allow_low_precision(reason="renormalizer feeds bf16 matmul anyway"):
            nc.vector.reciprocal(iv[:, :], pav_p[DH:DH + 1, :])
        invr[ph] = iv
    if h is None:
        return None
    if h != H - 1:
        nc.vector.reciprocal(invS[:, :], sT[:, :])
    pav = ps_at.tile([DH + 1, N], F32, tag="ps_av", bufs=1, name=f"pav{h}")
    return (h, pav, expT, invS, own_vhs)


def _make_vhs(nc, head, h, jt, v, invS, eng="pool"):
    vt = head.tile([P, DH + 1], CDT, tag=f"vhs{jt}", bufs=2, name=f"vhs{h}_{jt}")
    e = nc.vector if eng == "dve" else nc.gpsimd
    e.tensor_scalar_mul(vt[:, 0:DH], v[jt][:, h * DH:(h + 1) * DH],
                        invS[:, jt:jt + 1])
    e.tensor_copy(vt[:, DH:DH + 1], invS[:, jt:jt + 1])
    return vt


def _post_head(nc, norm, ps_at, done_h, outTu, outT, invr, sel_lo, sel_hi,
               woT, bo_b, y_acc, out_d):
    """Generator: after head `done_h` closes a pair, normalize the pair and
    accumulate its share of the output projection into y_acc. Driven at the
    filler budget so it never lumps against the head loop."""
    if done_h % 2 != 1:
        return
    et2 = done_h // 2
    for ic in range(2):
        pbc = ps_at.tile([P, 512], F32, tag="ps_misc", bufs=2,
                         name=f"pbc{et2}_{ic}")
        nc.tensor.matmul(pbc[:, :], sel_lo[:, :],
                         invr[2 * et2][:, ic * 512:(ic + 1) * 512],
                         start=True, stop=False)
        nc.tensor.matmul(pbc[:, :], sel_hi[:, :],
                         invr[2 * et2 + 1][:, ic * 512:(ic + 1) * 512],
                         start=False, stop=True)
        nc.vector.tensor_tensor(
            outT[et2][:, ic * 512:(ic + 1) * 512],
            outTu[et2][:, ic * 512:(ic + 1) * 512],
            pbc[:, :], Mult)
        yield
    for nt in range(8):
        py = ps_at.tile([P, D], F32, tag="ps_misc", bufs=2, name=f"py{et2}_{nt}")
        nc.tensor.matmul(py[:, :],
                         outT[et2][:, nt * P:(nt + 1) * P],
                         woT[:, et2 * D:(et2 + 1) * D],
                         start=True, stop=True)
        if et2 == 0:
            nc.vector.tensor_tensor(y_acc[nt][:, :], py[:, :], bo_b[:, :], Add)
        else:
            nc.vector.tensor_tensor(y_acc[nt][:, :], py[:, :], y_acc[nt][:, :], Add)
        yield


def _tail(nc, head, norm, ps_at, prev, v, outTu, outT, invr, sel_lo, sel_hi,
          woT, y_acc, out_d):
    """Head 7's attn@v with the two i-chunk groups de-interleaved so each
    chunk's drain -> reciprocal -> broadcast -> normalize -> project -> DMA
    chain overlaps the other chunk's matmuls."""
    ph, pav, pexp, pinvS, pvhs = prev
    for ic in range(2):
        for jt in range(8):
            nc.tensor.matmul(pav[:, ic * 512:(ic + 1) * 512],
                             pvhs[jt][:, :],
                             pexp[jt][:, ic * 512:(ic + 1) * 512],
                             start=(jt == 0), stop=(jt == 7))
        nc.scalar.copy(outTu[3][64:128, ic * 512:(ic + 1) * 512],
                       pav[0:DH, ic * 512:(ic + 1) * 512])
        iv = norm.tile([1, 512], CDT, tag="invr7", bufs=IB, name=f"invr7_{ic}")
        with nc.allow_low_precision(reason="renormalizer feeds bf16 matmul anyway"):
            nc.vector.reciprocal(iv[:, :], pav[DH:DH + 1, ic * 512:(ic + 1) * 512])
        pbc = ps_at.tile([P, 512], F32, tag="ps_misc", bufs=2, name=f"pbc3_{ic}")
        nc.tensor.matmul(pbc[:, :], sel_lo[:, :],
                         invr[6][:, ic * 512:(ic + 1) * 512],
                         start=True, stop=False)
        nc.tensor.matmul(pbc[:, :], sel_hi[:, :], iv[:, :],
                         start=False, stop=True)
        nc.vector.tensor_tensor(
            outT[3][:, ic * 512:(ic + 1) * 512],
            outTu[3][:, ic * 512:(ic + 1) * 512],
            pbc[:, :], Mult)
        for nt in range(4 * ic, 4 * ic + 4):
            py = ps_at.tile([P, D], F32, tag="ps_misc", bufs=2, name=f"py3_{nt}")
            nc.tensor.matmul(py[:, :],
                             outT[3][:, nt * P:(nt + 1) * P],
                             woT[:, 3 * D:4 * D],
                             start=True, stop=True)
            ysb = norm.tile([P, D], F32, tag="y", bufs=YB, name=f"y{nt}")
            nc.vector.tensor_tensor(ysb[:, :], py[:, :], y_acc[nt][:, :], Add)
            nc.sync.dma_start(out_d[nt * P:(nt + 1) * P, :], ysb[:, :])


_CACHE = {}


def get_nc():
    if "nc" not in _CACHE:
        # Bacc (not raw Bass): its compile() runs the wait-legalization passes
        # (move_matmul_waits_to_ldweights, generate_event_semaphores) that
        # walrus codegen requires (max 1 sync wait per instruction).
        nc = bacc.Bacc("TRN2", target_bir_lowering=False, num_devices=B)
        build(nc)
        nc.compile()
        _CACHE["nc"] = nc
    return _CACHE["nc"]


def kernel(x, context, Wq, Wk, Wv, Wo, bo):
    nc = get_nc()
    w = {
        "Wq": np.ascontiguousarray(Wq, dtype=np.float32),
        "Wk": np.ascontiguousarray(Wk, dtype=np.float32),
        "Wv": np.ascontiguousarray(Wv, dtype=np.float32),
        "Wo": np.ascontiguousarray(Wo, dtype=np.float32),
        "bo": np.ascontiguousarray(bo, dtype=np.float32),
    }
    in_maps = [
        {"x": np.ascontiguousarray(x[b], dtype=np.float32),
         "context": np.ascontiguousarray(context[b], dtype=np.float32),
         **w}
        for b in range(B)
    ]
    res = run_bass_kernel_spmd(nc, in_maps, core_ids=list(range(B)))
    _CACHE["last"] = res
    return np.stack([res.results[b]["out"] for b in range(B)], axis=0)

